# revision 21
# baseline (speedup 1.0000x reference)
"""Correlation cost-volume kernel for Trainium2 (Bass/Tile).

Problem: in1, in2: [B=8, C=128, H=96, W=128] fp32.
Output: [B, 81, H, W] where out[b, dy*9+dx, y, x] =
    mean_c( in1[b,c,y,x] * in2_pad[b,c,y+dy,x+dx] ),
with in2 zero-padded by 4 in both spatial dims (max_displacement=4).

Strategy (data-parallel over batch, one sample per NeuronCore):
  - Inputs cast to fp16 on host (PE full rate, half the load bytes;
    total error ~4e-3 rel vs the 2e-2 gate).
  - Per in1 row y: 3 TensorE matmuls compute the Gram
    G[x, (dy, x')] = sum_c in1[c,y,x] * in2p[c, y+dy, x'] into PSUM.
  - Scalar/Vector drain the per-32-partition-group 40-wide window
    wv[x, dy, u] = G[x, dy, 32*(x//32)+u] / C (fp16, contiguous —
    strided 2B engine writes are ~4x slower than contiguous, so the
    drain stays contiguous and the r-innermost relayout is a separate
    pass). Engine partition bases must be multiples of 32.
  - Row pairs are then relayouted into the r-innermost batch tile
    wt[x, dy, u, r] (GpSimd mostly — otherwise idle; pairing rows
    makes destination runs 4B). r-innermost matters because DMA cost
    is ~26ns per descriptor: with r innermost in both wt and t2 every
    extraction descriptor covers a 9*R*2-byte contiguous run, giving
    ~1k descriptors per batch vs ~37k for dx-innermost runs.
  - Once 32 rows are staged, 32 partition-strided SBUF->SBUF DMAs
    (s = x mod 32) extract the taps for all 32 rows at once:
    t2[x, dy, dx, r] = wt[x, dy, s+dx, r]. Mixed partition+byte
    strides in DMA APs miscompute on HW, so the per-s DMAs are the
    only legal way to apply the partition-dependent shift.
  - PE-transposes each row's [128 x, 81 k] band tile -> [81, 128]
    fp16 rows packed 8-per-PSUM-bank, Scalar/Vector stage 8-row
    blocks to SBUF, one DMA per 32-row batch stores the fp16 block.
    Host casts the gathered output to fp32.
"""

import numpy as np

import concourse.bass as bass
import concourse.mybir as mybir
from concourse import bacc
from concourse.bass_utils import run_bass_kernel_spmd
from concourse.tile import TileContext

B = 8
C = 128
H = 96
W = 128
D = 9  # 2*max_disp + 1
K = D * D  # 81 output channels
PAD = 4
WP = W + 2 * PAD  # 136
FP32 = mybir.dt.float32
FP16 = mybir.dt.float16

N_CORES = 8
R = 32  # rows per extraction batch (96 = 3 * 32)
TB = 8  # transposed rows packed per PSUM bank

NP_IN_DTYPE = np.float16


def _copy_on(nc, eng, dst, src):
    if eng is nc.scalar:
        nc.scalar.activation(dst, src, mybir.ActivationFunctionType.Copy)
    else:
        eng.tensor_copy(dst, src)


def _make_identity(nc, ident):
    # like masks.make_identity
    nc.gpsimd.memset(ident, 0.0)
    nc.gpsimd.affine_select(
        out=ident,
        in_=ident,
        compare_op=mybir.AluOpType.not_equal,
        fill=1.0,
        base=0,
        # out[x, y] = (x - y) != 0 ? in : fill
        pattern=[[-1, ident.shape[0]]],
        channel_multiplier=1,
    )


def build_bass(h: int = H):
    """Build the per-core Bass program for a [C, h, W] sample."""
    hp = h + 2 * PAD
    nb = h // R  # number of row batches
    assert h % R == 0 and R % TB == 0
    nc = bacc.Bacc(None, target_bir_lowering=False)
    in1 = nc.dram_tensor("in1", [C, h, W], FP16, kind="ExternalInput")
    # in2p is host-padded: [C, h+8, W+8] with zeros in the 4-wide borders.
    in2p = nc.dram_tensor("in2p", [C, hp, WP], FP16, kind="ExternalInput")
    out = nc.dram_tensor("out", [K, h, W], FP16, kind="ExternalOutput")

    with TileContext(nc) as tc:
        with (
            tc.tile_pool(name="big", bufs=1) as big_pool,
            tc.tile_pool(name="wvp", bufs=3) as wvp,
            tc.tile_pool(name="wtp", bufs=2) as wtp,
            tc.tile_pool(name="t2p", bufs=2) as t2p,
            tc.tile_pool(name="top", bufs=2) as top,
            tc.tile_pool(name="gpsum", bufs=2, space="PSUM") as gpsum,
            tc.tile_pool(name="tpsum", bufs=2, space="PSUM") as tpsum,
        ):
            s1 = big_pool.tile([C, h, W], FP16, name="s1")
            s2p = big_pool.tile([C, hp, WP], FP16, name="s2p")
            ident = big_pool.tile([128, 128], FP16, name="ident")
            _make_identity(nc, ident)

            # Load inputs in interleaved row-chunks so the first rows of
            # BOTH tensors land early and compute starts ~5us in.
            nchunk = 8
            rows1 = (h + nchunk - 1) // nchunk
            rows2 = (hp + nchunk - 1) // nchunk
            for ci in range(nchunk):
                i1 = ci * rows1
                r1 = min(rows1, h - i1)
                if r1 > 0:
                    nc.sync.dma_start(
                        s1[:, i1 : i1 + r1, :], in1[:, i1 : i1 + r1, :]
                    )
                i2 = ci * rows2
                r2 = min(rows2, hp - i2)
                if r2 > 0:
                    nc.sync.dma_start(
                        s2p[:, i2 : i2 + r2, :], in2p[:, i2 : i2 + r2, :]
                    )

            batches = [48, 40, 8] if h == 96 else [R] * nb
            assert sum(batches) == h
            y0 = 0
            for b, Rb in enumerate(batches):
                last = b == len(batches) - 1
                # wt[x, dy, u, r] = G[x, y0+r, dy, 32*(x//32)+u] / C
                wt = wtp.tile([128, D, 40, Rb], FP16, name="wt", tag="wt")
                wt_r = wt[:, :, :, :].rearrange(
                    "p (j rr) u r -> p j rr u r", j=3
                )
                wv = None
                for r in range(Rb):
                    y = y0 + r
                    # --- 3 matmuls: G[x, (dy, x')] over dy triplets ---
                    gp = gpsum.tile([128, 3, 512], FP32, name="gp", tag="gp")
                    for j in range(3):
                        nc.tensor.matmul(
                            gp[:, j, 0 : 3 * WP],
                            s1[:, y, :],
                            s2p[:, y + 3 * j : y + 3 * j + 3, :],
                            start=True,
                            stop=True,
                        )

                    # --- PSUM -> SBUF windowed drain (cast fp16, scale 1/C)
                    # contiguous, two rows per wv tile.
                    gp_r = gp[:, :, 0 : 3 * WP].rearrange(
                        "p j (rr n) -> p j rr n", rr=3
                    )
                    if r % 2 == 0:
                        wv = wvp.tile(
                            [128, 2, 3, 3, 40], FP16, name="wv", tag="wv"
                        )
                    for g in range(4):
                        src = gp_r[
                            32 * g : 32 * g + 32, :, :, 32 * g : 32 * g + 40
                        ]
                        dst = wv[32 * g : 32 * g + 32, r % 2, :, :, :]
                        if g % 2 == 0:
                            nc.scalar.activation(
                                dst,
                                src,
                                mybir.ActivationFunctionType.Copy,
                                scale=1.0 / C,
                            )
                        else:
                            nc.vector.tensor_scalar_mul(dst, src, 1.0 / C)

                    # --- relayout the row pair into the r-innermost batch
                    # tile; destination runs are the 2-row pairs (4B), which
                    # makes both sides 2-byte packed: DVE runs this in 2x
                    # mode (~543ns/pair vs 1042 scalar, 2562 pool).
                    if r % 2 == 1:
                        rp = r // 2
                        eng = nc.scalar if rp % 3 == 2 else nc.vector
                        # src iterated (j, rr, u, row2); dst pairs innermost
                        src = wv[:, :, :, :, :].rearrange(
                            "p r2 j rr u -> p j rr u r2"
                        )
                        _copy_on(
                            nc, eng, wt_r[:, :, :, :, r - 1 : r + 1], src
                        )

                # --- band extraction: 32 partition-strided SBUF->SBUF DMAs
                # covering the whole 32-row batch; 9*R*2B contiguous runs.
                # For s = x mod 32: t2[x, dy, dx, r] = wt[x, dy, s+dx, r]
                t2 = t2p.tile([128, D, D, Rb], FP16, name="t2", tag="t2")
                for s in range(32):
                    eng = nc.scalar if (last and s % 2 == 1) else nc.sync
                    eng.dma_start(
                        t2[s::32, :, :, :], wt[s::32, :, s : s + D, :]
                    )

                # --- per row: PE transpose band [128, 81] -> [81, 128],
                # TB rows packed per PSUM bank; stage each TB-block to SBUF.
                to = top.tile([128, Rb, W], FP16, name="to", tag="to")
                tt = None
                for r in range(Rb):
                    if r % TB == 0:
                        tt = tpsum.tile(
                            [128, TB, W], FP16, name="tt", tag="tt"
                        )
                    nc.tensor.transpose(
                        tt[0:K, r % TB, :], t2[:, :, :, r], ident[:, :]
                    )
                    if r % TB == TB - 1:
                        rb = r - (TB - 1)
                        eng = nc.scalar if (r // TB) % 2 == 0 else nc.vector
                        _copy_on(
                            nc, eng, to[0:K, rb : r + 1, :], tt[0:K, :, :]
                        )

                # --- store: one DMA per batch, partition k -> out[k, yb, :]
                nc.sync.dma_start(
                    out[:, y0 : y0 + Rb, :], to[0:K, :, :]
                )
                y0 += Rb

    nc.compile()
    return nc


_cached = {}


def _get_nc(h: int):
    if h not in _cached:
        _cached[h] = build_bass(h)
    return _cached[h]


def _pad_in2(in2: np.ndarray) -> np.ndarray:
    # [C, h, W] -> [C, h+8, W+8] zero-padded, contiguous
    return np.ascontiguousarray(
        np.pad(in2, ((0, 0), (PAD, PAD), (PAD, PAD)), mode="constant").astype(
            NP_IN_DTYPE, copy=False
        )
    )


def _make_in_maps(in1: np.ndarray, in2: np.ndarray):
    return [
        {
            "in1": np.ascontiguousarray(in1[b].astype(NP_IN_DTYPE, copy=False)),
            "in2p": _pad_in2(in2[b]),
        }
        for b in range(B)
    ]


def kernel(**inputs: np.ndarray) -> np.ndarray:
    in1 = np.asarray(inputs["in1"], dtype=np.float32)
    in2 = np.asarray(inputs["in2"], dtype=np.float32)
    assert in1.shape == (B, C, H, W), in1.shape

    nc = _get_nc(H)
    in_maps = _make_in_maps(in1, in2)
    res = run_bass_kernel_spmd(nc, in_maps, core_ids=list(range(N_CORES)))
    return np.stack(
        [r["out"].astype(np.float32) for r in res.results], axis=0
    )


# revision 24
# speedup vs baseline: 1.0874x; 1.0874x over previous
"""Correlation cost-volume kernel for Trainium2 (Bass/Tile).

Problem: in1, in2: [B=8, C=128, H=96, W=128] fp32.
Output: [B, 81, H, W] where out[b, dy*9+dx, y, x] =
    mean_c( in1[b,c,y,x] * in2_pad[b,c,y+dy,x+dx] ),
with in2 zero-padded by 4 in both spatial dims (max_displacement=4).

Strategy (data-parallel over batch, one sample per NeuronCore):
  - Inputs cast to fp16 on host (PE full rate, half the load bytes;
    total error ~4e-3 rel vs the 2e-2 gate).
  - Per in1 row y: 3 TensorE matmuls compute the Gram
    G[x, (dy, x')] = sum_c in1[c,y,x] * in2p[c, y+dy, x'] into PSUM.
  - Scalar/Vector drain the per-32-partition-group 40-wide window
    wv[x, dy, u] = G[x, dy, 32*(x//32)+u] / C (fp16, contiguous —
    strided 2B engine writes are ~4x slower than contiguous, so the
    drain stays contiguous and the r-innermost relayout is a separate
    pass). Engine partition bases must be multiples of 32.
  - Row pairs are then relayouted into the r-innermost batch tile
    wt[x, dy, u, r] (GpSimd mostly — otherwise idle; pairing rows
    makes destination runs 4B). r-innermost matters because DMA cost
    is ~26ns per descriptor: with r innermost in both wt and t2 every
    extraction descriptor covers a 9*R*2-byte contiguous run, giving
    ~1k descriptors per batch vs ~37k for dx-innermost runs.
  - Once 32 rows are staged, 32 partition-strided SBUF->SBUF DMAs
    (s = x mod 32) extract the taps for all 32 rows at once:
    t2[x, dy, dx, r] = wt[x, dy, s+dx, r]. Mixed partition+byte
    strides in DMA APs miscompute on HW, so the per-s DMAs are the
    only legal way to apply the partition-dependent shift.
  - PE-transposes each row's [128 x, 81 k] band tile -> [81, 128]
    fp16 rows packed 8-per-PSUM-bank, Scalar/Vector stage 8-row
    blocks to SBUF, one DMA per 32-row batch stores the fp16 block.
    Host casts the gathered output to fp32.
"""

import numpy as np

import concourse.bass as bass
import concourse.mybir as mybir
from concourse import bacc
from concourse.bass_utils import run_bass_kernel_spmd
from concourse.tile import TileContext

B = 8
C = 128
H = 96
W = 128
D = 9  # 2*max_disp + 1
K = D * D  # 81 output channels
PAD = 4
WP = W + 2 * PAD  # 136
FP32 = mybir.dt.float32
FP16 = mybir.dt.float16

N_CORES = 8
R = 32  # rows per extraction batch (96 = 3 * 32)
TB = 8  # transposed rows packed per PSUM bank

NP_IN_DTYPE = np.float16


def _copy_on(nc, eng, dst, src):
    if eng is nc.scalar:
        nc.scalar.activation(dst, src, mybir.ActivationFunctionType.Copy)
    else:
        eng.tensor_copy(dst, src)


def _make_identity(nc, ident):
    # like masks.make_identity
    nc.gpsimd.memset(ident, 0.0)
    nc.gpsimd.affine_select(
        out=ident,
        in_=ident,
        compare_op=mybir.AluOpType.not_equal,
        fill=1.0,
        base=0,
        # out[x, y] = (x - y) != 0 ? in : fill
        pattern=[[-1, ident.shape[0]]],
        channel_multiplier=1,
    )


def build_bass(h: int = H):
    """Build the per-core Bass program for a [C, h, W] sample."""
    hp = h + 2 * PAD
    nb = h // R  # number of row batches
    assert h % R == 0 and R % TB == 0
    nc = bacc.Bacc(None, target_bir_lowering=False)
    in1 = nc.dram_tensor("in1", [C, h, W], FP16, kind="ExternalInput")
    # in2p is host-padded: [C, h+8, W+8] with zeros in the 4-wide borders.
    in2p = nc.dram_tensor("in2p", [C, hp, WP], FP16, kind="ExternalInput")
    out = nc.dram_tensor("out", [K, h, W], FP16, kind="ExternalOutput")

    with TileContext(nc) as tc:
        with (
            tc.tile_pool(name="big", bufs=1) as big_pool,
            tc.tile_pool(name="wvp", bufs=4) as wvp,
            tc.tile_pool(name="wtp", bufs=2) as wtp,
            tc.tile_pool(name="t2p", bufs=2) as t2p,
            tc.tile_pool(name="top", bufs=2) as top,
            tc.tile_pool(name="gpsum", bufs=2, space="PSUM") as gpsum,
            tc.tile_pool(name="tpsum", bufs=2, space="PSUM") as tpsum,
        ):
            s1 = big_pool.tile([C, h, W], FP16, name="s1")
            s2p = big_pool.tile([C, hp, WP], FP16, name="s2p")
            ident = big_pool.tile([128, 128], FP16, name="ident")
            _make_identity(nc, ident)

            # Load inputs in interleaved row-chunks so the first rows of
            # BOTH tensors land early and compute starts ~5us in.
            nchunk = 8
            rows1 = (h + nchunk - 1) // nchunk
            rows2 = (hp + nchunk - 1) // nchunk
            for ci in range(nchunk):
                i1 = ci * rows1
                r1 = min(rows1, h - i1)
                if r1 > 0:
                    nc.sync.dma_start(
                        s1[:, i1 : i1 + r1, :], in1[:, i1 : i1 + r1, :]
                    )
                i2 = ci * rows2
                r2 = min(rows2, hp - i2)
                if r2 > 0:
                    nc.sync.dma_start(
                        s2p[:, i2 : i2 + r2, :], in2p[:, i2 : i2 + r2, :]
                    )

            batches = [R] * nb
            assert sum(batches) == h
            y0 = 0
            for b, Rb in enumerate(batches):
                last = b == len(batches) - 1
                # wt[x, dy, u, r] = G[x, y0+r, dy, 32*(x//32)+u] / C
                wt = wtp.tile([128, D, 40, Rb], FP16, name="wt", tag="wt")
                wt_r = wt[:, :, :, :].rearrange(
                    "p (j rr) u r -> p j rr u r", j=3
                )
                wv = None
                for r in range(Rb):
                    y = y0 + r
                    # --- 3 matmuls: G[x, (dy, x')] over dy triplets ---
                    gp = gpsum.tile([128, 3, 512], FP32, name="gp", tag="gp")
                    for j in range(3):
                        nc.tensor.matmul(
                            gp[:, j, 0 : 3 * WP],
                            s1[:, y, :],
                            s2p[:, y + 3 * j : y + 3 * j + 3, :],
                            start=True,
                            stop=True,
                        )

                    # --- PSUM -> SBUF windowed drain (cast fp16, scale 1/C)
                    # contiguous, two rows per wv tile.
                    gp_r = gp[:, :, 0 : 3 * WP].rearrange(
                        "p j (rr n) -> p j rr n", rr=3
                    )
                    if r % 2 == 0:
                        wv = wvp.tile(
                            [128, 2, 3, 3, 40], FP16, name="wv", tag="wv"
                        )
                    for g in range(4):
                        src = gp_r[
                            32 * g : 32 * g + 32, :, :, 32 * g : 32 * g + 40
                        ]
                        dst = wv[32 * g : 32 * g + 32, r % 2, :, :, :]
                        if g % 2 == 0:
                            nc.scalar.activation(
                                dst,
                                src,
                                mybir.ActivationFunctionType.Copy,
                                scale=1.0 / C,
                            )
                        else:
                            nc.vector.tensor_scalar_mul(dst, src, 1.0 / C)

                    # --- relayout the row pair into the r-innermost batch
                    # tile; destination runs are the 2-row pairs (4B), which
                    # makes both sides 2-byte packed: DVE runs this in 2x
                    # mode (~543ns/pair vs 1042 scalar, 2562 pool).
                    # All relayouts go to Pool: putting them on the drain
                    # engines stalls their in-order queues behind
                    # cross-engine waits (measured slower overall even
                    # though Pool's copy itself is 2.5x slower than DVE's).
                    if r % 2 == 1:
                        src = wv[:, :, :, :, :].rearrange(
                            "p r2 j rr u -> p j rr u r2"
                        )
                        nc.gpsimd.tensor_copy(
                            wt_r[:, :, :, :, r - 1 : r + 1], src
                        )

                # --- band extraction: 32 partition-strided SBUF->SBUF DMAs
                # covering the whole 32-row batch; 9*R*2B contiguous runs.
                # For s = x mod 32: t2[x, dy, dx, r] = wt[x, dy, s+dx, r]
                t2 = t2p.tile([128, D, D, Rb], FP16, name="t2", tag="t2")
                for s in range(32):
                    eng = nc.scalar if (last and s % 2 == 1) else nc.sync
                    eng.dma_start(
                        t2[s::32, :, :, :], wt[s::32, :, s : s + D, :]
                    )

                # --- per row: PE transpose band [128, 81] -> [81, 128],
                # TB rows packed per PSUM bank; stage each TB-block to SBUF.
                to = top.tile([128, Rb, W], FP16, name="to", tag="to")
                tt = None
                for r in range(Rb):
                    if r % TB == 0:
                        tt = tpsum.tile(
                            [128, TB, W], FP16, name="tt", tag="tt"
                        )
                    nc.tensor.transpose(
                        tt[0:K, r % TB, :], t2[:, :, :, r], ident[:, :]
                    )
                    if r % TB == TB - 1:
                        rb = r - (TB - 1)
                        eng = nc.scalar if (r // TB) % 2 == 0 else nc.vector
                        _copy_on(
                            nc, eng, to[0:K, rb : r + 1, :], tt[0:K, :, :]
                        )

                # --- store: one DMA per batch, partition k -> out[k, yb, :]
                nc.sync.dma_start(
                    out[:, y0 : y0 + Rb, :], to[0:K, :, :]
                )
                y0 += Rb

    nc.compile()
    return nc


_cached = {}


def _get_nc(h: int):
    if h not in _cached:
        _cached[h] = build_bass(h)
    return _cached[h]


def _pad_in2(in2: np.ndarray) -> np.ndarray:
    # [C, h, W] -> [C, h+8, W+8] zero-padded, contiguous
    return np.ascontiguousarray(
        np.pad(in2, ((0, 0), (PAD, PAD), (PAD, PAD)), mode="constant").astype(
            NP_IN_DTYPE, copy=False
        )
    )


def _make_in_maps(in1: np.ndarray, in2: np.ndarray):
    return [
        {
            "in1": np.ascontiguousarray(in1[b].astype(NP_IN_DTYPE, copy=False)),
            "in2p": _pad_in2(in2[b]),
        }
        for b in range(B)
    ]


def kernel(**inputs: np.ndarray) -> np.ndarray:
    in1 = np.asarray(inputs["in1"], dtype=np.float32)
    in2 = np.asarray(inputs["in2"], dtype=np.float32)
    assert in1.shape == (B, C, H, W), in1.shape

    nc = _get_nc(H)
    in_maps = _make_in_maps(in1, in2)
    res = run_bass_kernel_spmd(nc, in_maps, core_ids=list(range(N_CORES)))
    return np.stack(
        [r["out"].astype(np.float32) for r in res.results], axis=0
    )
